# revision 20
# baseline (speedup 1.0000x reference)
"""Trainium2 Bass kernel for nn_MultiHeadAttention_45019847196962.

Reference computation (per batch b):
    q = Q @ Wq + bq                 # (Lq, H*D)
    v = V @ Wv + bv                 # (Lk, H*D)   (used as both keys and values)
    scores = q_h @ v_h^T            # per head, no 1/sqrt(d) scale
    align  = softmax(scores, -1)
    attn   = align @ v_h            # concat heads -> (Lq, H*D)
    out    = tanh([attn | Q] @ Wf + bf)

Sharding: data-parallel over batch. 16 batches / 8 cores = 2 batches per
core; weights replicated. No collectives.

Key algebraic restructuring vs the obvious dataflow:
  - bv is dropped on-device entirely: softmax rows are shift-invariant so
    dropping bv from the keys changes nothing, and align rows sum to one
    so align@(v+bv) = align@v + bv, which folds into the fc bias as
    bf' = bf + bv @ Wf[:H*D].  This removes all value-bias DVE work.
  - bq and bf' are added via K=1 rank-1 accumulation matmuls into the
    same PSUM group as the projection / fc, so no separate bias pass.
  - The softmax denominator S rides as a 65th "ones" column on the attn
    matmul stationary (align rows summing to 1 make this exact).
  - attn normalization (x 1/S) happens on the DVE as a tensor-tensor
    multiply against a PE-broadcast r row, fused with the PSUM drain.

Scheduling (the critical insight): exp on the Scalar engine is a hard
~71us/core floor (8.4M elems at ~1.1ns/col), and warm-clock PE work is
~80us/core, so the kernel must keep BOTH saturated.  Batch 1's
projections and batch 0's fc run as PE "filler" interleaved between
batch 0/1's exp-bound attention slots.  Startup casts are chunked and
the input transposes are split across both HWDGE rings (sync + scalar)
so the first projection starts ~8us in.
"""

import numpy as np

B, LQ, LK = 16, 512, 1024
F, H, D = 512, 8, 64
NCORES = 8
BPC = B // NCORES  # batches per core

_CACHE = {}


def _split_sync_waits(nc, mybir, maxw=1):
    """This container's walrus rejects instructions with more than one sync
    wait ("Too many sync wait commands").  Move excess waits onto NoOp
    instructions inserted just before the over-subscribed instruction on the
    same engine queue (program order preserves the wait semantics)."""
    for fn in nc.m.functions:
        for blk in fn.blocks:
            insts = blk.instructions
            i = 0
            while i < len(insts):
                inst = insts[i]
                si = getattr(inst, "sync_info", None)
                if si is not None and len(si.on_wait) > maxw:
                    waits = list(si.on_wait)
                    del si.on_wait[maxw:]
                    pre = []
                    for j in range(maxw, len(waits), maxw):
                        nop = mybir.InstNoOp(
                            name=nc.get_next_instruction_name(),
                            engine=inst.engine,
                            ins=[],
                            outs=[],
                            sync_info=mybir.SyncInfo(
                                on_wait=waits[j:j + maxw], on_update=[]),
                        )
                        pre.append(nop)
                    insts[i:i] = pre
                    i += len(pre)
                i += 1


def _patch_sem_clear_chunking(bass, chunk=16):
    """walrus here rejects the kernel-tail SEM_RANGE_CLEAR ISA op when the
    semaphore range is large ("ISA wrong length").  Chunk the ranges."""
    if getattr(bass.Bass.clear_and_free_semaphores, "_chunked", False):
        return
    orig = bass.Bass.clear_and_free_semaphores

    def chunked(self, sems):
        sems = list(sems)
        nums = [s.num if hasattr(s, "num") else s for s in sems]
        order = sorted(range(len(sems)), key=lambda i: nums[i])
        for j in range(0, len(sems), chunk):
            orig(self, [sems[i] for i in order[j:j + chunk]])

    chunked._chunked = True
    bass.Bass.clear_and_free_semaphores = chunked


def _build():
    import concourse.bass as bass
    import concourse.tile as tile
    from concourse import mybir

    _patch_sem_clear_chunking(bass)

    dt = mybir.dt
    f32, bf16 = dt.float32, dt.bfloat16
    AF = mybir.ActivationFunctionType
    OP = mybir.AluOpType

    nc = bass.Bass("TRN2", target_bir_lowering=False, debug=False,
                   num_devices=NCORES)

    Qd = nc.dram_tensor("Q", [BPC, LQ, F], f32, kind="ExternalInput").ap()
    Vd = nc.dram_tensor("V", [BPC, LK, F], f32, kind="ExternalInput").ap()
    Wqd = nc.dram_tensor("Wq", [F, H * D], f32, kind="ExternalInput").ap()
    bqd = nc.dram_tensor("bq", [H * D], f32, kind="ExternalInput").ap()
    Wvd = nc.dram_tensor("Wv", [F, H * D], f32, kind="ExternalInput").ap()
    bvd = nc.dram_tensor("bv", [H * D], f32, kind="ExternalInput").ap()
    Wfd = nc.dram_tensor("Wf", [F + H * D, F], f32, kind="ExternalInput").ap()
    bfd = nc.dram_tensor("bf", [F], f32, kind="ExternalInput").ap()
    Od = nc.dram_tensor("O", [BPC, LQ, F], f32, kind="ExternalOutput").ap()

    Qbf = nc.dram_tensor("Qbf", [BPC, LQ, F], bf16).ap()
    # V bf16 staging, flattened to [batch*half] so the DRAM-slice deps keep
    # each half's cast -> transpose chain independent.
    Vbf = nc.dram_tensor("Vbf", [BPC * 2, LK // 2, F], bf16).ap()

    with tile.TileContext(nc) as tc:
        import contextlib
        with contextlib.ExitStack() as ctx:
            def pool(name, bufs, space="SBUF"):
                return ctx.enter_context(
                    tc.tile_pool(name=name, bufs=bufs, space=space))

            const_p = pool("const", 1)
            qt_p = pool("qt", 2)        # Q^T (bf16 input transpose)
            vt_p = pool("vt", 4)        # V^T, one tile per (batch, lk-half)
            qproj_p = pool("qproj", 2)  # qT
            vproj_p = pool("vproj", 2)  # vT
            vn_p = pool("vn", 2)        # v natural (+ones col)
            e_p = pool("E", 2)          # exp(scores^T) per pair
            at_p = pool("attnT", 2)
            s_p = pool("s_sb", 3)
            au_p = pool("au", 5)
            s4_p = pool("s4", 3)
            r4_p = pool("r4", 3)
            r0_p = pool("r0", 3)
            rbc_p = pool("rbc", 3)
            ao_p = pool("anodd", 3)
            osb_p = pool("osb", 3)

            # PSUM: 8 banks of [128, 512] f32.
            ps_sc = pool("ps_sc", 2, space="PSUM")  # scores [128,2,512]: 4
            ps_at = pool("ps_at", 2, space="PSUM")  # attn out [128,512]: 2
            ps_f = pool("ps_f", 2, space="PSUM")    # proj/fc/misc: 2

            # ---- startup DMA: SWDGE casts in need-order (V0 half 0 first
            # so the big V-dependent projections can start earliest).  The
            # transposed loads ride the sync HWDGE ring in the same order
            # so they pipeline behind the casts. ----
            nc.gpsimd.dma_start(Vbf[0], Vd[0][0:512])
            nc.gpsimd.dma_start(Qbf[0], Qd[0])
            nc.gpsimd.dma_start(Vbf[1], Vd[0][512:1024])
            nc.gpsimd.dma_start(Vbf[2], Vd[1][0:512])
            nc.gpsimd.dma_start(Vbf[3], Vd[1][512:1024])
            nc.gpsimd.dma_start(Qbf[1], Qd[1])
            Wf_sb = const_p.tile([128, 8, F], bf16)
            nc.gpsimd.dma_start(
                Wf_sb[:], Wfd.rearrange("(ko p) n -> p ko n", p=128))

            # scalar HWDGE ring: weights + bias rows, needed first.
            wst_p = ctx.enter_context(tc.tile_pool(name="wstage", bufs=1))
            Wq_f32 = wst_p.tile([128, 4, H * D], f32, name="wstage",
                                tag="wstage")
            nc.scalar.dma_start(
                Wq_f32[:], Wqd.rearrange("(ko p) n -> p ko n", p=128))
            Wq_sb = const_p.tile([128, 4, H * D], bf16)
            nc.vector.tensor_copy(Wq_sb[:], Wq_f32[:])
            Wv_f32 = wst_p.tile([128, 4, H * D], f32, name="wstage",
                                tag="wstage")
            nc.scalar.dma_start(
                Wv_f32[:], Wvd.rearrange("(ko p) n -> p ko n", p=128))
            Wv_sb = const_p.tile([128, 4, H * D], bf16)
            nc.vector.tensor_copy(Wv_sb[:], Wv_f32[:])

            bq_f32 = const_p.tile([1, H * D], f32)
            nc.sync.dma_start(bq_f32[:], bqd.rearrange("(a n) -> a n", a=1))
            bq_row = const_p.tile([1, H * D], bf16)
            nc.vector.tensor_copy(bq_row[:], bq_f32[:])
            bf_row = const_p.tile([1, F], f32)
            nc.sync.dma_start(bf_row[:], bfd.rearrange("(a n) -> a n", a=1))
            bv_f32 = const_p.tile([128, 4], f32)
            nc.sync.dma_start(
                bv_f32[:], bvd.rearrange("(ko p) -> p ko", p=128))
            bv_col = const_p.tile([128, 4], bf16)
            nc.vector.tensor_copy(bv_col[:], bv_f32[:])

            ones_sb = const_p.tile([1, 64], bf16)
            nc.vector.memset(ones_sb[:], 1.0)
            ones_row = const_p.tile([1, F], bf16)
            nc.vector.memset(ones_row[:], 1.0)

            # ---- transposed input loads.  NOTE: DMA transposes on the
            # scalar (Act) HWDGE ring produce corrupted data on this HW
            # (timing-dependent, verified by bisection) — the XBAR
            # transpose path is only reliable on the sync ring.  Plain
            # DMAs on the scalar ring are fine.  VT is split into separate
            # per-half tiles: Tile dependencies are tile-granular, so a
            # single [128,4,LK] tile would stall the first projection
            # until ALL eight transposes land. ----
            QTs = []
            VTh = []  # VTh[2*b+h]: [128, 4, 512] = V^T lk-half h of batch b
            for b in range(BPC):
                QTs.append(qt_p.tile([128, 4, LQ], bf16, name="QT",
                                     tag="QT"))
                VTh.append(vt_p.tile([128, 4, LK // 2], bf16, name="VTh",
                                     tag="VTh"))
                VTh.append(vt_p.tile([128, 4, LK // 2], bf16, name="VTh",
                                     tag="VTh"))

            def emit_vt_half(b, h):
                for ko in range(4):
                    nc.sync.dma_start(
                        VTh[2 * b + h][:, ko, :],
                        Vbf[2 * b + h][:, ko * 128:(ko + 1) * 128],
                        transpose=True)

            def emit_qt(b):
                for ko in range(4):
                    nc.sync.dma_start(
                        QTs[b][:, ko, :],
                        Qbf[b][:, ko * 128:(ko + 1) * 128],
                        transpose=True)

            emit_vt_half(0, 0)
            emit_qt(0)
            emit_vt_half(0, 1)
            emit_vt_half(1, 0)
            emit_vt_half(1, 1)
            emit_qt(1)

            # bf' = bf + bv @ Wf[:H*D]  (the folded value-bias correction)
            bfp_row = const_p.tile([1, F], bf16)

            def emit_bfp():
                ps = ps_f.tile([128, 512], f32, name="psf", tag="psf")
                for k in range(4):
                    nc.tensor.matmul(ps[0:1, :], bv_col[:, k:k + 1],
                                     Wf_sb[:, k, :], start=(k == 0),
                                     stop=(k == 3))
                nc.vector.tensor_tensor(bfp_row[:], ps[0:1, :], bf_row[:],
                                        op=OP.add)

            # ---- per-batch state ----
            qTs = [None, None]
            vTs = [None, None]
            vns = [None, None]
            attnTs = [None, None]

            # PE work units (each: ~1-2us of matmuls + off-PE drain).
            # Emitted inline during a batch's own phase or popped as
            # filler between exp-bound attention slots.
            def unit_vt_proj(b, n, m):
                def emit():
                    ps = ps_f.tile([128, 512], f32, name="psf", tag="psf")
                    for kk in range(4):
                        nc.tensor.matmul(
                            ps[:], Wv_sb[:, kk, m * 128:(m + 1) * 128],
                            VTh[2 * b + n][:, kk, :],
                            start=(kk == 0), stop=(kk == 3))
                    nc.vector.tensor_copy(
                        vTs[b][:, m, n * 512:(n + 1) * 512], ps[:])
                return emit

            def unit_vn_proj(b, c):
                def emit():
                    ps = ps_f.tile([128, 512], f32, name="psf", tag="psf")
                    for kk in range(4):
                        nc.tensor.matmul(
                            ps[:],
                            VTh[2 * b + c // 4][:, kk,
                                                (c % 4) * 128:
                                                (c % 4 + 1) * 128],
                            Wv_sb[:, kk, :], start=(kk == 0), stop=(kk == 3))
                    nc.vector.tensor_copy(
                        vns[b][:, c, :, 0:64],
                        ps[:].rearrange("p (h d) -> p h d", d=64))
                return emit

            def unit_q_proj(b, m):
                def emit():
                    ps = ps_f.tile([128, 512], f32, name="psf", tag="psf")
                    for kk in range(4):
                        nc.tensor.matmul(
                            ps[:], Wq_sb[:, kk, m * 128:(m + 1) * 128],
                            QTs[b][:, kk, :], start=(kk == 0), stop=False)
                    # rank-1 bias: qT[m-chunk, :] += bq[m-chunk] * ones
                    nc.tensor.matmul(
                        ps[:], bq_row[:, m * 128:(m + 1) * 128],
                        ones_row[:], start=False, stop=True)
                    nc.vector.tensor_copy(qTs[b][:, m, :], ps[:])
                return emit

            def fc_q_part(b, m):
                ps = ps_f.tile([128, 512], f32, name="psf", tag="psf")
                # bias row first (start clears the bank)
                nc.tensor.matmul(ps[:], ones_row[:, 0:128], bfp_row[:],
                                 start=True, stop=False)
                for kk in range(4):
                    nc.tensor.matmul(
                        ps[:], QTs[b][:, kk, m * 128:(m + 1) * 128],
                        Wf_sb[:, kk + 4, :], start=False, stop=False)
                return ps

            def fc_attn_part(b, m, ps):
                for kk in range(4):
                    nc.tensor.matmul(
                        ps[:], attnTs[b][:, kk, m * 128:(m + 1) * 128],
                        Wf_sb[:, kk, :], start=False, stop=(kk == 3))
                osb = osb_p.tile([128, 512], f32, name="osb", tag="osb")
                nc.scalar.activation(osb[:], ps[:], AF.Tanh)
                # output stores ride the (otherwise idle) SWDGE queue so
                # the sync ring stays free for the 1/S chain hops
                nc.gpsimd.dma_start(
                    Od[b][m * 128:(m + 1) * 128, :], osb[:])

            def unit_fc(b, m):
                def emit():
                    ps = fc_q_part(b, m)
                    fc_attn_part(b, m, ps)
                return emit

            def proj_units(b):
                us = []
                for n in (0,):
                    for m in range(4):
                        us.append(unit_vt_proj(b, n, m))
                for c in range(4):
                    us.append(unit_vn_proj(b, c))
                for m in range(4):
                    us.append(unit_q_proj(b, m))
                for n in (1,):
                    for m in range(4):
                        us.append(unit_vt_proj(b, n, m))
                for c in range(4, 8):
                    us.append(unit_vn_proj(b, c))
                return us

            # ---- attention machinery ----
            # Scores/exp pipeline per head-pair p: per lk-chunk c one psS
            # [128,2,512] (head A bank 0, head B bank 1) -> ONE exp
            # instruction into E[:, :, c, :].  The attn matmuls (M=65 with
            # the ones column) accumulate per head into ps_at, drained to
            # au/s2 by the DVE.  The 1/S chain is pipelined 2 pairs deep.
            def make_stage_B(st):
                st["r8"] = r4_p.tile([128, 8], f32, name="r8", tag="r8")
                nc.vector.reciprocal(st["r8"][:], st["s8"][:])
                st["r8b"] = r4_p.tile([128, 8], bf16, name="r8b", tag="r8b")
                nc.vector.tensor_copy(st["r8b"][:], st["r8"][:])
                st["r2"] = r0_p.tile([1, 2, 512], bf16, name="r2", tag="r2")
                nc.sync.dma_start(st["r2"][:], st["r8b"][:])

            def make_stage_C(st, attnT):
                for s in range(2):
                    psR = ps_at.tile([128, 512], f32, name="psAt", tag="at")
                    nc.tensor.matmul(psR[0:64, :], ones_sb[:],
                                     st["r2"][0:1, s, :],
                                     start=True, stop=True)
                    rbc = rbc_p.tile([64, 512], f32, name="rbc", tag="rbc")
                    nc.vector.tensor_copy(rbc[:], psR[0:64, :])
                    if s == 0:
                        nc.vector.tensor_tensor(
                            attnT[0:64, st["p"], :], st["au"][s][:],
                            rbc[:], op=OP.mult)
                    else:
                        an = ao_p.tile([64, 512], bf16, name="an", tag="an")
                        nc.vector.tensor_tensor(
                            an[:], st["au"][s][:], rbc[:], op=OP.mult)
                        nc.sync.dma_start(
                            attnT[64:128, st["p"], :], an[:])

            def attn_batch(b, fillers):
                """Emit the attention phase for batch b.  `fillers` is a
                list of PE work-unit closures popped between exp-gated
                slots (batch 1's projections during batch 0's phase,
                batch 0's fc during batch 1's phase)."""
                qT, vT, vn = qTs[b], vTs[b], vns[b]
                attnT = attnTs[b]
                chain = []

                def pop_filler(k=1):
                    for _ in range(k):
                        if fillers:
                            fillers.pop(0)()

                def scores_chunk(p, c, E):
                    psS = ps_sc.tile([128, 2, 512], f32, name="psS",
                                     tag="sc")
                    nc.tensor.matmul(
                        psS[:, 0, :], vT[0:64, p, c * 128:(c + 1) * 128],
                        qT[0:64, p, :], start=True, stop=True)
                    nc.tensor.matmul(
                        psS[:, 1, :], vT[64:128, p, c * 128:(c + 1) * 128],
                        qT[64:128, p, :], start=True, stop=True)
                    nc.scalar.activation(E[:, :, c, :], psS[:], AF.Exp)

                def attn_head(p, s, E, st):
                    h = 2 * p + s
                    psAt = ps_at.tile([128, 512], f32, name="psAt",
                                      tag="at")
                    for c in range(8):
                        nc.tensor.matmul(
                            psAt[0:65, :], vn[:, c, h, 0:65], E[:, s, c, :],
                            start=(c == 0), stop=(c == 7))
                    nc.vector.tensor_copy(st["s2"][64:65, s, :],
                                          psAt[64:65, :])
                    au = au_p.tile([64, 512], f32, name="au", tag="au")
                    nc.vector.tensor_copy(au[:], psAt[0:64, :])
                    st["au"][s] = au

                Es = []
                sts = []
                # prologue: scores for pair 0
                E0 = e_p.tile([128, 2, 8, 512], bf16, name="E", tag="E")
                for c in range(8):
                    scores_chunk(0, c, E0)
                    if c in (2, 5):
                        pop_filler()
                Es.append(E0)

                for p in range(4):
                    E = Es[p]
                    st = {"p": p, "au": [None, None]}
                    st["s2"] = s_p.tile([65, 2, 512], f32, name="s2",
                                        tag="s2")
                    sts.append(st)
                    if p < 3:
                        En = e_p.tile([128, 2, 8, 512], bf16, name="E",
                                      tag="E")
                        Es.append(En)
                        # interleave next pair's scores with this pair's
                        # attn matmuls; both are exp-gated so filler units
                        # absorb the PE slack.
                        for c in range(8):
                            scores_chunk(p + 1, c, En)
                            if c == 1:
                                attn_head(p, 0, E, st)
                                pop_filler()
                            elif c == 4:
                                attn_head(p, 1, E, st)
                                pop_filler()
                            elif c == 6:
                                pop_filler()
                    else:
                        attn_head(p, 0, E, st)
                        pop_filler()
                        attn_head(p, 1, E, st)
                        pop_filler()
                    st["s8"] = s4_p.tile([128, 8], f32, name="s8", tag="s8")
                    nc.sync.dma_start(st["s8"][:], st["s2"][64:65, :, :])
                    chain.append(st)
                    if len(chain) >= 2:
                        make_stage_B(chain[-2])
                    if len(chain) >= 3:
                        make_stage_C(chain[-3], attnT)
                        pop_filler()
                # flush the normalize pipeline
                # flush of the last two pairs' normalize chain is returned
                # as closures so the caller can overlap it with the next
                # batch's prologue (the chain has ~3us of DMA latency).
                def flush_a():
                    make_stage_B(chain[-1])
                    make_stage_C(chain[-2], attnT)

                def flush_b():
                    make_stage_C(chain[-1], attnT)
                return [flush_a, flush_b]

            # ================= emission =================
            for b in range(BPC):
                qTs[b] = qproj_p.tile([128, 4, LQ], bf16, name="qT",
                                      tag="qT")
                vTs[b] = vproj_p.tile([128, 4, LK], bf16, name="vT",
                                      tag="vT")
                vns[b] = vn_p.tile([128, 8, 8, 68], bf16, name="vn",
                                   tag="vn")
                attnTs[b] = at_p.tile([128, 4, LQ], bf16, name="attnT",
                                      tag="attnT")
                nc.vector.memset(vns[b][:, :, :, 64:65], 1.0)

            # PE warmup: dummy K=1 matmuls during the initial DMA phase so
            # the HAM clock gate is at 8/8 when the real work arrives, and
            # the PE never shows a >3.4us idle window at the start.
            ps_warm = ps_f.tile([128, 512], f32, name="psf", tag="psf")
            for _ in range(16):
                nc.tensor.matmul(ps_warm[0:64, :], ones_sb[:],
                                 ones_row[:], start=True, stop=True)

            # batch 0 projections inline (nothing else to overlap with)
            for u in proj_units(0):
                u()

            # batch 0 attention; batch 1 projections as filler
            fill = proj_units(1)
            flush0 = attn_batch(0, fill)
            # any un-popped b1 projections must be emitted before b1's
            # attention reads them (Tile deps follow program order)
            while fill:
                fill.pop(0)()
            emit_bfp()
            # batch 1 attention; batch 0's normalize flush + fc as filler
            fill = flush0 + [unit_fc(0, m) for m in range(4)]
            flush1 = attn_batch(1, fill)
            while fill:
                fill.pop(0)()
            # tail: pre-accumulate fc Q-parts to cover the last pairs'
            # 1/S chain latency, then finish per-chunk as attnT lands.
            ps0 = fc_q_part(1, 0)
            ps1 = fc_q_part(1, 1)
            flush1[0]()
            flush1[1]()
            fc_attn_part(1, 0, ps0)
            ps2 = fc_q_part(1, 2)
            fc_attn_part(1, 1, ps1)
            ps3 = fc_q_part(1, 3)
            fc_attn_part(1, 2, ps2)
            fc_attn_part(1, 3, ps3)

    _split_sync_waits(nc, mybir)
    return nc


def _get_nc():
    if "nc" not in _CACHE:
        _CACHE["nc"] = _build()
    return _CACHE["nc"]


def kernel(Q, V, Wq, bq, Wv, bv, Wf, bf, _trace=False):
    from concourse.bass_utils import run_bass_kernel_spmd

    nc = _get_nc()
    Q = np.ascontiguousarray(np.asarray(Q, dtype=np.float32))
    V = np.ascontiguousarray(np.asarray(V, dtype=np.float32))
    shared = {
        "Wq": np.ascontiguousarray(np.asarray(Wq, np.float32)),
        "bq": np.ascontiguousarray(np.asarray(bq, np.float32)),
        "Wv": np.ascontiguousarray(np.asarray(Wv, np.float32)),
        "bv": np.ascontiguousarray(np.asarray(bv, np.float32)),
        "Wf": np.ascontiguousarray(np.asarray(Wf, np.float32)),
        "bf": np.ascontiguousarray(np.asarray(bf, np.float32)),
    }
    in_maps = []
    for c in range(NCORES):
        m = {"Q": Q[c * BPC:(c + 1) * BPC], "V": V[c * BPC:(c + 1) * BPC]}
        m.update(shared)
        in_maps.append(m)

    res = run_bass_kernel_spmd(nc, in_maps, core_ids=list(range(NCORES)),
                               trace=_trace)
    out = np.concatenate([res.results[c]["O"] for c in range(NCORES)], axis=0)
    if _trace:
        _CACHE["last_exec_time_ns"] = res.exec_time_ns
        _CACHE["last_res"] = res
    return out
